# revision 78
# baseline (speedup 1.0000x reference)
# Multi-head causal attention (b=2, n=2048, dim=1024, 16 heads) on 8 TRN2
# NeuronCores. Sharding: core c -> batch c//4, head-group c%4 (4 heads = 256
# channels). Host pre-transposes x and the weight slices (all bf16) so every
# device-side matmul contracts over the partition dimension.
#
# Device-side design (per core):
#   xT   [1024, 2048] bf16  x[b].T              (streamed in 512-col chunks)
#   QT/KT [128, 2, 2048] bf16, two heads packed per partition block; the S
#        matmul contracts K=64 partitions at offset (h%2)*64 — no zero-fill
#   Vp   [128, 16, 4, 72] bf16 V in [j, head, d] layout, row pitch 144B so
#        every [jb,h] row is 16B-aligned (HW requirement for the moving
#        operand); col 64 = ones
#   S^T  [j, i] tiles in psum; diagonal units pack both j-block spans
#        contiguously so ONE exp instruction covers them; exp -> bf16 es
#   AV   runs TRANSPOSED: lhsT = 128x128 es block (weights), rhs = Vp slice
#        => pO [i, d+1] at 65 rows/matmul (half the PE rows of the [d, i]
#        orientation); the ones column accumulates the softmax denominator
#        per-PARTITION; all four i-blocks share one psum bank via a single
#        lazily-zeroed accumulation group
#   norm reciprocal (DVE) + per-partition tensor_scalar_mul -> ATn [i, chan]
#   ATn  -> AT [chan, i] via dma_start_transpose (xbar: out[p,hp,i] =
#        in[i, hp*128+p]); the last chunk uses PE transpose-mode instead
#        (psS bank is free by then) to stay off the serial HWDGE path
#   Wo   lhsT = AT block, rhs = wo_s -> psum -> bf16 ob pair-tiles -> one
#        merged DMA per 256 output rows (f32 partials summed on host)
# Scheduling: one continuous S->exp->AV software pipeline across (ic, h);
# projections for chunk ic+1 and the output projection fill PE slack. The
# output-projection units of chunks 0-2 are DEFERRED into the Act-bound
# final chunk (WO_DEFER) where exp pacing would otherwise starve the PE.
# Causal masking: diagonal 128-blocks are exp'd raw then multiplied by a
# 0/1 triangle on gpsimd (off the DVE queue, whose head-of-line order
# gates psum recycling); fully-masked spans are never computed.
# Softmax skips max-subtraction (scores are O(+-10); exp is safe in fp32).
import os

if os.environ.get("JAX_PLATFORMS") == "cpu":
    # bass2jax must see the axon/neuron PJRT devices.
    del os.environ["JAX_PLATFORMS"]

from contextlib import ExitStack

import numpy as np

import concourse.bass as bass
import concourse.bacc as bacc
import concourse.mybir as mybir
import concourse.tile as tile
from concourse import bass_utils

F32 = mybir.dt.float32
BF16 = mybir.dt.bfloat16
AF = mybir.ActivationFunctionType

P = 128
SEQ = 2048
DIM = 1024
CH = 256          # channels per core (4 heads x 64)
HD = 64           # head dim
VW = 72           # Vp row pitch: 144B so every [j,h] row is 16B-aligned
NH = 4            # heads per core
KO = DIM // P     # 8 contraction chunks
NIC = SEQ // 512  # 4 i-chunks
SCALE = float(HD) ** -0.5


def build_nc():
    nc = bacc.Bacc("TRN2", target_bir_lowering=False, debug=False, num_devices=8)
    xT = nc.dram_tensor("xT", [DIM, SEQ], BF16, kind="ExternalInput").ap()
    wqT = nc.dram_tensor("wqT", [DIM, CH], BF16, kind="ExternalInput").ap()
    wkT = nc.dram_tensor("wkT", [DIM, CH], BF16, kind="ExternalInput").ap()
    wvT = nc.dram_tensor("wvT", [DIM, CH], BF16, kind="ExternalInput").ap()
    woT = nc.dram_tensor("woT", [CH, DIM], BF16, kind="ExternalInput").ap()
    out = nc.dram_tensor("out", [SEQ, DIM], BF16, kind="ExternalOutput").ap()

    with ExitStack() as ctx:
        tc = ctx.enter_context(tile.TileContext(nc))
        per = ctx.enter_context(tc.tile_pool(name="persist", bufs=1))
        wpool = ctx.enter_context(tc.tile_pool(name="wts", bufs=1))
        xpool = ctx.enter_context(tc.tile_pool(name="xch", bufs=3))
        espool = ctx.enter_context(tc.tile_pool(name="es", bufs=6))
        atnpool = ctx.enter_context(tc.tile_pool(name="atn", bufs=8))
        rpool = ctx.enter_context(tc.tile_pool(name="rcp", bufs=4))
        opool = ctx.enter_context(tc.tile_pool(name="osb", bufs=10))
        psS = ctx.enter_context(tc.tile_pool(name="psS", bufs=2, space="PSUM"))
        psO = ctx.enter_context(tc.tile_pool(name="psO", bufs=2, space="PSUM"))
        psP = ctx.enter_context(tc.tile_pool(name="psP", bufs=2, space="PSUM"))

        QT = per.tile([P, 2, SEQ], BF16)
        KT = per.tile([P, 2, SEQ], BF16)
        Vp = per.tile([P, SEQ // P, NH, VW], BF16)
        AT = per.tile([P, 2, SEQ], BF16)
        maskT = per.tile([P, P], BF16)   # triangle: 1 where i >= j
        ident = per.tile([P, P], BF16)   # identity for PE transpose mode
        mstage = per.tile([P, P], F32)
        scr = per.tile([P, 512], F32)
        scrb = per.tile([P, 512], BF16)

        wq_s = wpool.tile([P, KO, CH], BF16)
        wk_s = wpool.tile([P, KO, CH], BF16)
        wv_s = wpool.tile([P, KO, CH], BF16)
        wo_s = wpool.tile([P, 2, DIM], BF16)

        xch = {}

        def load_x(ic, split=1):
            t = xpool.tile([P, KO, 512], BF16, tag="xch", name=f"xch{ic}")
            src = xT[:, ic * 512:(ic + 1) * 512].rearrange("(ko p) f -> p ko f", p=P)
            kstep = KO // split
            for s in range(split):
                nc.sync.dma_start(
                    t[:, s * kstep:(s + 1) * kstep, :],
                    src[:, s * kstep:(s + 1) * kstep, :],
                )
            xch[ic] = t

        # DMA order matters for startup latency: the first projection needs
        # only wq + xch0, quartered so the ko-loop can start on the first
        # quarter.
        wq_src = wqT.rearrange("(ko p) c -> p ko c", p=P)
        x0t = xpool.tile([P, KO, 512], BF16, tag="xch", name="xch0")
        x0src = xT[:, 0:512].rearrange("(ko p) f -> p ko f", p=P)
        for s in range(4):
            # interleave wq and x0 quarters: the first projection matmul
            # needs (wq ko=0..1, x0 ko=0..1) — don't make x0 wait behind
            # the whole of wq in the serial DMA pipe
            nc.scalar.dma_start(
                wq_s[:, 2 * s:2 * s + 2, :], wq_src[:, 2 * s:2 * s + 2, :]
            )
            nc.sync.dma_start(
                x0t[:, 2 * s:2 * s + 2, :], x0src[:, 2 * s:2 * s + 2, :]
            )
        xch[0] = x0t
        nc.sync.dma_start(wk_s[:], wkT.rearrange("(ko p) c -> p ko c", p=P))
        # wv before x1: the prologue V projection is on the DMA critical path
        nc.sync.dma_start(wv_s[:], wvT.rearrange("(ko p) c -> p ko c", p=P))
        # warmup scratch on the DVE queue (empty at t=0; Pool starts with the
        # framework const memsets): the first warm matmul gates on it
        nc.vector.memset(scrb[:], 0.0)
        # ones column of Vp: the AV matmul then also accumulates the softmax
        # denominator into pO col 64
        nc.gpsimd.memset(Vp[:, :, :, HD], 1.0)
        load_x(1)
        nc.sync.dma_start(wo_s[:], woT.rearrange("(co p) f -> p co f", p=P))

        # PE warmup: dummy bf16 matmuls on scratch ramp the clock while the
        # first DMAs land, without delaying the first projection much.
        wps = psP.tile([P, 512], F32, tag="psP", name="warm")
        for _ in range(5):
            nc.tensor.matmul(
                wps[:], lhsT=scrb[:, 0:P], rhs=scrb[:], start=True, stop=True
            )

        # preload the Exp activation table while Act is idle; the first real
        # exp would otherwise pay the ~1.3us table load on the critical path
        nc.gpsimd.memset(scr[0:1, 0:2], 0.0)
        nc.scalar.activation(scr[0:1, 0:2], scr[0:1, 0:2], AF.Exp)

        # causal mask for diagonal 128-blocks: keep (1.0) where i >= j
        nc.gpsimd.memset(mstage[:], 1.0)
        nc.gpsimd.affine_select(
            out=mstage[:],
            in_=mstage[:],
            compare_op=mybir.AluOpType.is_ge,
            fill=0.0,
            base=0,
            channel_multiplier=-1,
            pattern=[[1, P]],
        )
        nc.gpsimd.tensor_copy(maskT[:], mstage[:])

        # identity matrix for PE transpose-mode (used for the last chunk's
        # AT blocks where the DMA xbar path would serialize on HWDGE)
        nc.gpsimd.memset(mstage[:], 1.0)
        nc.gpsimd.affine_select(
            out=mstage[:],
            in_=mstage[:],
            compare_op=mybir.AluOpType.is_equal,
            fill=0.0,
            base=0,
            channel_multiplier=-1,
            pattern=[[1, P]],
        )
        nc.gpsimd.tensor_copy(ident[:], mstage[:])

        def proj_mm_unit(ic, co, w_s, name):
            ps = psP.tile([P, 512], F32, tag="psP", name=f"p{name}{ic}{co}")
            for ko in range(KO):
                nc.tensor.matmul(
                    ps[:],
                    lhsT=w_s[:, ko, co * P:(co + 1) * P],
                    rhs=xch[ic][:, ko, :],
                    start=(ko == 0),
                    stop=(ko == KO - 1),
                )
            return ps

        def _cp(eng):
            return nc.scalar.copy if eng == 'act' else nc.vector.tensor_copy

        def proj_qk_evac(dst, ic, co, ps, eng=None):
            i0 = ic * 512
            _cp(eng)(dst[:, co, i0:i0 + 512], ps[:])

        def proj_v_mm(ic, g):
            ps = psP.tile([P, 512], F32, tag="psP", name=f"pv{ic}{g}")
            for u in range(2):
                for ko in range(KO):
                    nc.tensor.matmul(
                        ps[:, u * 256:(u + 1) * 256],
                        lhsT=xch[ic][:, ko, (2 * g + u) * P:(2 * g + u + 1) * P],
                        rhs=wv_s[:, ko, :],
                        start=(ko == 0),
                        stop=(ko == KO - 1),
                    )
            return ps

        def proj_v_evac(ic, g, ps, eng=None):
            _cp(eng)(
                Vp[:, 4 * ic + 2 * g:4 * ic + 2 * g + 2, :, 0:HD],
                ps[:].rearrange("p (j h d) -> p j h d", j=2, h=NH),
            )

        def proj_fillers(ic):
            st = {}
            units = []
            for co in range(2):
                units.append(lambda co=co: st.__setitem__(
                    ('q', co), proj_mm_unit(ic, co, wq_s, 'q')))
                units.append(lambda co=co: proj_qk_evac(QT, ic, co, st.pop(('q', co))))
            for co in range(2):
                units.append(lambda co=co: st.__setitem__(
                    ('k', co), proj_mm_unit(ic, co, wk_s, 'k')))
                units.append(lambda co=co: proj_qk_evac(KT, ic, co, st.pop(('k', co))))
            for g in range(2):
                units.append(lambda g=g: st.__setitem__(('v', g), proj_v_mm(ic, g)))
                units.append(lambda g=g: proj_v_evac(ic, g, st.pop(('v', g))))
            return units

        # ---- prologue: full projection for i-chunk 0 on the Act engine ----
        for co in range(2):
            proj_qk_evac(QT, 0, co, proj_mm_unit(0, co, wq_s, 'q'), eng='act')
        for co in range(2):
            proj_qk_evac(KT, 0, co, proj_mm_unit(0, co, wk_s, 'k'), eng='act')
        for g in range(2):
            proj_v_evac(0, g, proj_v_mm(0, g), eng='act')
        load_x(2)
        load_x(3)

        # ---- one continuous S -> exp -> AV pipeline across every (ic, h) ----
        S_units = []
        for ic in range(NIC):
            for h in range(NH):
                for t in range(2 * ic + 2):
                    S_units.append((ic, h, t))

        es_tiles = {}
        pO_tiles = {}
        atn_tiles = {}
        ob_tiles = {}
        proj_fq = []   # projection units: must drain before the next ic
        wo_fq = []     # (due_pos, fn) output-projection units
        tr_fq = []     # transpose units: emitted eagerly (DMA queue, cheap)
        delayed = []   # (due_pos, fn)
        WO_DEFER = 46  # hold ic<3 wo matmuls for the Act-bound ic=3 stretch

        def spans(ic, t):
            # per-u (base, off): the u-span of the unit lives at psum/es cols
            # [base, base + 512 - off) and covers i-offsets [off, 512). Diag
            # units pack both spans contiguously so ONE exp instruction covers
            # them (halves the Act per-instruction overhead on the diagonal).
            r0, r1 = 2 * t - 4 * ic, 2 * t + 1 - 4 * ic
            off0 = 0 if r0 < 0 else P * r0
            off1 = 0 if r1 < 0 else P * r1
            if t >= 2 * ic:
                return [(0, off0), (512 - off0, off1)]
            return [(0, 0), (512, 0)]

        def emit_S(ic, h, t):
            co = h // 2
            hp = (h % 2) * HD
            diag = t >= 2 * ic
            sp = spans(ic, t)
            pS = psS.tile([P, 1024], F32, tag="psS", name=f"pS{ic}{h}{t}")
            for u in range(2):
                jb = 2 * t + u
                base, off = sp[u]
                nc.tensor.matmul(
                    pS[:, base:base + 512 - off],
                    lhsT=KT[hp:hp + HD, co, jb * P:(jb + 1) * P],
                    rhs=QT[hp:hp + HD, co, ic * 512 + off:(ic + 1) * 512],
                    start=True,
                    stop=True,
                )
            es = espool.tile([P, 1024], BF16, tag="es", name=f"es{ic}{h}{t}")
            end = sp[1][0] + 512 - sp[1][1]
            nc.scalar.activation(
                es[:, 0:end] if diag else es[:],
                pS[:, 0:end] if diag else pS[:],
                AF.Exp,
                scale=SCALE,
            )
            if diag:
                # the first 128 cols of each u-span are its diagonal block;
                # apply the triangular causal mask there (gpsimd: all-SBUF op,
                # keeps the psS recycle path off the DVE queue; the very last
                # head goes to DVE instead — the Pool-queue hop would delay
                # the final AV matmuls on the tail critical path)
                mul = nc.vector.tensor_mul
                for u in range(2):
                    base = sp[u][0]
                    mul(
                        es[:, base:base + P],
                        es[:, base:base + P],
                        maskT[:],
                    )
            es_tiles[(ic, h, t)] = es

        def norm_pair(ic, h, pr, pO, pos):
            def fn():
                rcp = rpool.tile([P, 2], F32, tag="rcp", name=f"r{ic}{h}{pr}")
                nc.vector.reciprocal(rcp[:], pO[:, 2 * pr:2 * pr + 2, HD])
                for k in range(2):
                    ib = 2 * pr + k
                    key = (ic, ib)
                    if key not in atn_tiles:
                        atn_tiles[key] = atnpool.tile(
                            [P, CH], BF16, tag="atn", name=f"atn{ic}{ib}"
                        )
                    if ic == NIC - 1 and h == NH - 1 and k == 0:
                        # tail: Act is done with exps — split the two muls
                        # across Act and DVE to shorten the critical chain
                        nc.scalar.activation(
                            atn_tiles[key][:, h * HD:(h + 1) * HD],
                            pO[:, ib, 0:HD],
                            AF.Copy,
                            scale=rcp[:, k:k + 1],
                        )
                    else:
                        nc.vector.tensor_scalar_mul(
                            atn_tiles[key][:, h * HD:(h + 1) * HD],
                            pO[:, ib, 0:HD],
                            rcp[:, k:k + 1],
                        )
                if h == NH - 1:
                    tr_fq.append((pos + 2, transpose_unit(ic, pr)))
                    due = max(pos + 3, WO_DEFER) if ic < NIC - 1 else pos + 3
                    for k in range(2):
                        io = 4 * ic + 2 * pr + k
                        for fc in range(2):
                            wo_fq.append((due, wo_unit(io, fc)))
                if pr == 1:
                    pO_tiles.pop((ic, h))
            return fn

        def transpose_unit(ic, pr):
            def fn():
                if ic == NIC - 1:
                    # tail: PE transpose mode (psS is free by now) — the DMA
                    # xbar path would serialize 625ns/transpose on HWDGE right
                    # on the critical path
                    pt = psS.tile([P, 4, P], BF16, tag="psS", name=f"pt{pr}")
                    tls = [atn_tiles.pop((ic, 2 * pr + k)) for k in range(2)]
                    for k in range(2):
                        for hp2 in range(2):
                            j = 2 * k + hp2
                            nc.tensor.matmul(
                                pt[:, j, :],
                                lhsT=tls[k][:, hp2 * P:(hp2 + 1) * P],
                                rhs=ident[:],
                                is_transpose=True,
                                start=(j == 0),
                                stop=(j == 3),
                            )
                    for k in range(2):
                        gi = 4 * ic + 2 * pr + k
                        for hp2 in range(2):
                            j = 2 * k + hp2
                            cp = nc.scalar.copy if j % 2 else nc.vector.tensor_copy
                            cp(AT[:, hp2, gi * P:(gi + 1) * P], pt[:, j, :])
                    return
                for k in range(2):
                    ib = 2 * pr + k
                    gi = 4 * ic + ib
                    t_ = atn_tiles.pop((ic, ib))
                    # xbar semantics: out[p, hp, i] = in[i, hp*128 + p]
                    nc.sync.dma_start_transpose(
                        AT[:, :, gi * P:(gi + 1) * P], t_[:]
                    )
            return fn

        def wo_unit(io, fc):
            def emit(eng=None, pool=None):
                pool = pool or psP
                ps2 = pool.tile(
                    [P, 512], F32, tag="psP" if pool is psP else "psS",
                    name=f"po{io}{fc}"
                )
                for co2 in range(2):
                    nc.tensor.matmul(
                        ps2[:],
                        lhsT=AT[:, co2, io * P:(io + 1) * P],
                        rhs=wo_s[:, co2, fc * 512:(fc + 1) * 512],
                        start=(co2 == 0),
                        stop=(co2 == 1),
                    )
                iop = io // 2
                if iop not in ob_tiles:
                    # one [P, 2, DIM] tile per io-pair: a single merged DMA
                    # per 256 output rows halves the HWDGE dispatch slots
                    ob_tiles[iop] = opool.tile(
                        [P, 2, DIM], BF16, tag="ob", name=f"ob{iop}"
                    )
                _cp(eng)(
                    ob_tiles[iop][:, io % 2, fc * 512:(fc + 1) * 512], ps2[:]
                )
                if iop >= 6 and fc == 1:
                    # the very last pair ships per-io so the first transfer
                    # overlaps the second io's compute
                    nc.sync.dma_start(
                        out[io * P:(io + 1) * P, :], ob_tiles[iop][:, io % 2, :]
                    )
                    if io % 2 == 1:
                        ob_tiles.pop(iop)
                elif io % 2 == 1 and fc == 1:
                    nc.sync.dma_start(
                        out[iop * 256:(iop + 1) * 256, :].rearrange(
                            "(a p) f -> p a f", p=P
                        ),
                        ob_tiles.pop(iop)[:],
                    )
            return emit

        def emit_AV(pos, ic, h, t):
            key = (ic, h)
            if key not in pO_tiles:
                pO_tiles[key] = psO.tile(
                    [P, NIC, HD + 1], F32, tag="psO", name=f"pO{ic}{h}"
                )
            pO = pO_tiles[key]
            es = es_tiles.pop((ic, h, t))
            sp = spans(ic, t)
            # One psum accumulation group for the whole [P, 4, 65] tile: the
            # 2KB zero region is bank-wide, so per-ib groups cannot interleave.
            # start marks the region pending-zero; each ib's first touch then
            # lazily zeroes its span.
            for u in range(2):
                jb = 2 * t + u
                base, off = sp[u]
                for ib in range(NIC):
                    gi = 4 * ic + ib
                    if gi < jb:
                        continue
                    c0 = base + ib * P - off
                    nc.tensor.matmul(
                        pO[:, ib, :],
                        lhsT=es[:, c0:c0 + P],
                        rhs=Vp[:, jb, h, 0:HD + 1],
                        start=(t == 0 and u == 0 and ib == 0),
                        stop=(t == 2 * ic + 1 and u == 1 and ib == 3),
                    )
            if t == 2 * ic:
                delayed.append((pos + 2, norm_pair(ic, h, 0, pO, pos)))
            elif t == 2 * ic + 1:
                delayed.append((pos + 2, norm_pair(ic, h, 1, pO, pos)))

        def run_due(pos):
            while delayed and delayed[0][0] <= pos:
                delayed.pop(0)[1]()

        npos = len(S_units)
        for pos, (ic, h, t) in enumerate(S_units):
            if t == 0 and h == 0:
                while proj_fq:  # safety: next ic's inputs must exist by now
                    proj_fq.pop(0)()
                if ic + 1 < NIC:
                    proj_fq.extend(proj_fillers(ic + 1))
            emit_S(ic, h, t)
            run_due(pos)
            if pos >= 2:
                emit_AV(pos, *S_units[pos - 2])
            while tr_fq and tr_fq[0][0] <= pos:
                tr_fq.pop(0)[1]()
            if t == 2 * ic + 1:  # head's S-units done -> slip in fillers
                for _ in range(3):
                    if proj_fq:
                        proj_fq.pop(0)()
            if pos >= WO_DEFER and pos % 4 != 0:
                # 2-per-3-positions drip of wo units: 24 deferred units then
                # last exactly to the end of the Act-bound final chunk
                if wo_fq and wo_fq[0][0] <= pos:
                    wo_fq.pop(0)[1]()
        emit_AV(npos, *S_units[npos - 2])
        run_due(npos)
        emit_AV(npos + 1, *S_units[npos - 1])
        run_due(npos + 10)
        while tr_fq:
            tr_fq.pop(0)[1]()
        tail = 0
        while wo_fq:
            wo_fq.pop(0)[1](
                eng='act' if tail % 2 == 0 else None,
                pool=psS if tail % 2 == 0 else psP,
            )
            tail += 1

    nc.compile()
    return nc


_NC = None


def get_nc():
    global _NC
    if _NC is None:
        _NC = build_nc()
    return _NC


def make_in_maps(x, Wq, Wk, Wv, Wo):
    from ml_dtypes import bfloat16

    x = np.asarray(x, dtype=np.float32)
    Wq = np.asarray(Wq, dtype=np.float32)
    Wk = np.asarray(Wk, dtype=np.float32)
    Wv = np.asarray(Wv, dtype=np.float32)
    Wo = np.asarray(Wo, dtype=np.float32)
    in_maps = []
    for c in range(8):
        b, g = divmod(c, 4)
        hs = g * CH
        in_maps.append(
            {
                "xT": np.ascontiguousarray(x[b].T).astype(bfloat16),
                "wqT": np.ascontiguousarray(Wq[hs:hs + CH, :].T).astype(bfloat16),
                "wkT": np.ascontiguousarray(Wk[hs:hs + CH, :].T).astype(bfloat16),
                "wvT": np.ascontiguousarray(Wv[hs:hs + CH, :].T).astype(bfloat16),
                "woT": np.ascontiguousarray(Wo[:, hs:hs + CH].T).astype(bfloat16),
            }
        )
    return in_maps


LAST_RESULTS = None


def kernel(x, Wq, Wk, Wv, Wo, trace=False):
    global LAST_RESULTS
    nc = get_nc()
    in_maps = make_in_maps(x, Wq, Wk, Wv, Wo)
    res = bass_utils.run_bass_kernel_spmd(
        nc, in_maps, core_ids=list(range(8)), trace=trace
    )
    LAST_RESULTS = res
    partials = [np.asarray(r["out"], dtype=np.float32) for r in res.results]
    out0 = partials[0] + partials[1] + partials[2] + partials[3]
    out1 = partials[4] + partials[5] + partials[6] + partials[7]
    return np.stack([out0, out1])
